# revision 47
# baseline (speedup 1.0000x reference)
"""Trainium2 Bass kernel for nn_Attention_59785944760577 (sparse_attention).

reference math per batch sample (B=8, one sample per NeuronCore):
  s[t]   = w2 . tanh(x[t] @ W1 + b1) + b2
  e[t]   = exp(s[t])            (softmax shift cancels in the num/den ratio)
  ctx[t] = cumsum_t(e * x) / cumsum_t(e)

v5 pipeline — scores in transposed layout, cumsum via e-scaled triangular
matmuls in natural layout, cross-tile carry + den + divide on the HOST:
  1. host uploads xT bf16 [128p, 8tb, 4dc, 512t] and x-natural tiles
     xn bf16 [128j, 32m, 512d]  (t = 128 m + j)
  2. hT[e,t] = tanh(sum_d W1[d,e] xT[d,t])      PE (W1 chunks stationary)
  3. s[1,t]  = sum_e w2[e] hT[e,t]              PE K-reduction matmul
     e_row   = exp(s + b2)  (fp32)              ACT, single partition
  4. e-cols: PE transpose of e_row 128-slices -> pEcol PSUM -> ecols SBUF
  5. U_e[m]  = U * e_col[m]  (DVE tensor_scalar, bf16)
     pN[m]   = U_e[m]^T xn[m]                   PE: local cumsum of e*x
  6. evict pN -> bf16 (DVE/GPSIMD), DMA out; ship e_row too
  7. HOST: carry[m] = cumsum of tile totals (= row 127 of each tile,
     already in the output), den = cumsum(bf16(e)), out=(local+carry)/den,
     reshaped to [T, D] fp32.
"""
import json
from contextlib import ExitStack

import numpy as np

import concourse.bass as bass
import concourse.tile as tile
from concourse import mybir
from concourse.bass_utils import run_bass_kernel_spmd
from concourse.vector_clock import ScopedClock

F32 = mybir.dt.float32
BF16 = mybir.dt.bfloat16
AF = mybir.ActivationFunctionType
ALU = mybir.AluOpType

B, T, D = 8, 4096, 512
P = 128
NTB = 8          # t-blocks of 512 for the scores phase
TB = T // NTB    # 512
NC_ = 4          # d/e chunks of 128
NM = T // P      # 32 cumsum tiles of 128 rows
N_CORES = 8
N_GPS_EVICT = 12  # how many of the 32 evicts go to gpsimd


# --- workarounds for this walrus build: at most ONE semaphore wait per
# instruction (see baseline notes).
def _patched_drain_and_barrier(self, tick_clock, wait_clock):
    nc = self.nc
    drain_inst = nc.sync.drain()
    wait_clock.add_sem_waits(
        drain_inst.ins, ScopedClock({None: tick_clock.global_clock})
    )
    si = drain_inst.ins.sync_info
    if si is not None and si.on_wait and len(si.on_wait) > 1:
        waits = list(si.on_wait)
        drain_inst.ins.sync_info = mybir.SyncInfo(
            on_wait=waits[:1], on_update=list(si.on_update)
        )
        for w in waits[1:]:
            extra = nc.sync.drain()
            extra.ins.sync_info = mybir.SyncInfo(on_wait=[w], on_update=[])
    nc.all_engine_barrier()
    assert self.sems is not None
    popped = nc._tile_sem_poison_stack.pop()
    assert popped is self._sem_poison
    nc.clear_and_free_semaphores(list(self.sems.allocated().values()))
    nc.all_engine_barrier()


def _split_multiwait_json(data: bytes) -> bytes:
    d = json.loads(data)
    changed = False
    for fn in d.get("functions", []):
        for bb in fn.get("blocks", []):
            new_insts = []
            for inst in bb.get("instructions", []):
                si = inst.get("sync_info")
                waits = si.get("on_wait") if si else None
                if waits and len(waits) > 1:
                    for k, w in enumerate(waits[:-1]):
                        new_insts.append(
                            {
                                "debug": inst.get("debug", 0),
                                "engine": inst["engine"],
                                "ins": [],
                                "outs": [],
                                "name": f"{inst['name']}-ws{k}",
                                "opcode": "NoOp",
                                "sync_info": {"on_update": [], "on_wait": [w]},
                            }
                        )
                    si["on_wait"] = [waits[-1]]
                    changed = True
                new_insts.append(inst)
            if changed:
                bb["instructions"] = new_insts
    return json.dumps(d).encode() if changed else data


def _install_patches():
    if not getattr(tile.TileContext, "_drain_patched", False):
        tile.TileContext._drain_and_barrier = _patched_drain_and_barrier
        tile.TileContext._drain_patched = True
    if not getattr(bass.Bass, "_json_waitsplit_patched", False):
        orig = bass.Bass.to_json_bytes

        def to_json_bytes(self):
            return _split_multiwait_json(orig(self))

        bass.Bass.to_json_bytes = to_json_bytes
        bass.Bass._json_waitsplit_patched = True


def build_nc(b2: float = 0.0):
    _install_patches()
    nc = bass.Bass()
    xT_d = nc.dram_tensor("xT", [P, NTB, NC_, TB], BF16, kind="ExternalInput")
    xn_d = nc.dram_tensor("xn", [P, NM, D], BF16, kind="ExternalInput")
    w1_d = nc.dram_tensor("w1", [P, NC_, NC_, P], BF16, kind="ExternalInput")
    w2_d = nc.dram_tensor("w2", [P, NC_], BF16, kind="ExternalInput")
    u_d = nc.dram_tensor("u128", [P, P], BF16, kind="ExternalInput")
    num_d = nc.dram_tensor("num", [P, NM, D], BF16, kind="ExternalOutput")
    e_d = nc.dram_tensor("e", [1, T], F32, kind="ExternalOutput")

    with tile.TileContext(nc) as tc, ExitStack() as ctx:
        consts = ctx.enter_context(tc.tile_pool(name="consts", bufs=1))
        xpool = ctx.enter_context(tc.tile_pool(name="x", bufs=1))
        hpool = ctx.enter_context(tc.tile_pool(name="h", bufs=2))
        spool = ctx.enter_context(tc.tile_pool(name="s", bufs=1))
        uepool = ctx.enter_context(tc.tile_pool(name="ue", bufs=9))
        obpool = ctx.enter_context(tc.tile_pool(name="ob", bufs=4))
        # PSUM (8 banks): H 3 + S 1 + Ecol 1 + N 3
        psH = ctx.enter_context(tc.tile_pool(name="psH", bufs=3, space="PSUM"))
        psS = ctx.enter_context(tc.tile_pool(name="psS", bufs=1, space="PSUM"))
        psE = ctx.enter_context(tc.tile_pool(name="psE", bufs=1, space="PSUM"))
        psN = ctx.enter_context(tc.tile_pool(name="psN", bufs=3, space="PSUM"))

        w1_sb = consts.tile([P, NC_, NC_, P], BF16, tag="w1")
        w2_sb = consts.tile([P, NC_], BF16, tag="w2")
        u_sb = consts.tile([P, P], BF16, tag="u")
        i11 = consts.tile([1, 1], F32, tag="i11")
        xT = xpool.tile([P, NTB, NC_, TB], BF16, tag="xT")
        xn = xpool.tile([P, NM, D], BF16, tag="xn")

        def emit_dma_xT(tb0, ntb):
            nc.sync.dma_start(xT[:, tb0 : tb0 + ntb], xT_d[:, tb0 : tb0 + ntb])

        def emit_dma_xn(tb0, ntb):
            nc.sync.dma_start(
                xn[:, 4 * tb0 : 4 * (tb0 + ntb), :],
                xn_d[:, 4 * tb0 : 4 * (tb0 + ntb), :],
            )

        # dispatch order tuned for ramp: first-needed chunks first
        # (w1 is ec-major so [:, 0] covers the first ec block's 4 dc chunks)
        nc.sync.dma_start(w1_sb[:, 0], w1_d[:, 0])
        nc.sync.dma_start(xT[:, 0, 0:1], xT_d[:, 0, 0:1])
        nc.sync.dma_start(xT[:, 0, 1:2], xT_d[:, 0, 1:2])
        nc.sync.dma_start(xT[:, 0, 2:4], xT_d[:, 0, 2:4])
        nc.sync.dma_start(w1_sb[:, 1:4], w1_d[:, 1:4])
        nc.sync.dma_start(xT[:, 1], xT_d[:, 1])
        nc.sync.dma_start(w2_sb[:], w2_d[:])
        nc.sync.dma_start(u_sb[:], u_d[:])
        nc.vector.memset(i11[:], 1.0)
        nc.sync.dma_start(xn[:, 0:4, :], xn_d[:, 0:4, :])
        nc.sync.dma_start(xn[:, 4:8, :], xn_d[:, 4:8, :])

        e_row = spool.tile([1, T], F32, tag="erow")
        ecols = spool.tile([P, NM], F32, tag="ecols")
        pEcol = psE.tile([P, NM], F32, tag="pecol")

        hts = [None] * NTB
        pss = [None] * NTB

        def emit_scores_mm(tb):
            ht = hpool.tile([P, NC_, TB], BF16)
            hts[tb] = ht
            for ec in range(NC_):
                ph = psH.tile([P, TB], F32)
                for dc in range(NC_):
                    nc.tensor.matmul(
                        ph[:],
                        w1_sb[:, ec, dc, :],
                        xT[:, tb, dc, :],
                        start=(dc == 0),
                        stop=(dc == NC_ - 1),
                    )
                nc.scalar.activation(ht[:, ec, :], ph[:], AF.Tanh)

        def emit_sred_exp(tb):
            ps = psS.tile([1, TB], F32)
            pss[tb] = ps
            ht = hts[tb]
            for ec in range(NC_):
                nc.tensor.matmul(
                    ps[:],
                    w2_sb[:, ec : ec + 1],
                    ht[:, ec, :],
                    start=(ec == 0),
                    stop=(ec == NC_ - 1),
                )
            nc.scalar.activation(
                e_row[0:1, tb * TB : (tb + 1) * TB], ps[:], AF.Exp, bias=float(b2)
            )

        def emit_etransp(tb):
            # 4 transposes: e_row [1,128] slices -> pEcol columns, then one evict
            for j in range(4):
                m = 4 * tb + j
                nc.tensor.transpose(
                    pEcol[:, m : m + 1], e_row[0:1, m * P : (m + 1) * P], i11[:]
                )
            nc.vector.tensor_copy(
                ecols[:, 4 * tb : 4 * tb + 4], pEcol[:, 4 * tb : 4 * tb + 4]
            )

        ue_tiles = {}

        def emit_ue_builds(tb):
            for j in range(4):
                m = 4 * tb + j
                ue = uepool.tile([P, P], BF16)
                nc.vector.tensor_scalar_mul(ue[:], u_sb[:], ecols[:, m : m + 1])
                ue_tiles[m] = ue

        def emit_cumsum(tb):
            last = tb == NTB - 1
            ob = obpool.tile([P, 4, D], BF16)
            for j in range(4):
                m = 4 * tb + j
                pn = psN.tile([P, D], F32)
                nc.tensor.matmul(
                    pn[:], ue_tiles.pop(m)[:], xn[:, m, :], start=True, stop=True
                )
                if last:
                    # split halves across ACT+DVE: half the evict latency
                    nc.scalar.copy(ob[:, j, 0 : D // 2], pn[:, 0 : D // 2])
                    nc.vector.tensor_copy(ob[:, j, D // 2 : D], pn[:, D // 2 : D])
                elif j % 2 == 0:
                    nc.scalar.copy(ob[:, j, :], pn[:])
                else:
                    nc.vector.tensor_copy(ob[:, j, :], pn[:])
                if last and j % 2 == 1:
                    nc.sync.dma_start(
                        num_d[:, m - 1 : m + 1, :], ob[:, j - 1 : j + 1, :]
                    )
            if not last:
                nc.sync.dma_start(num_d[:, 4 * tb : 4 * (tb + 1), :], ob[:])

        # software pipeline: exp(k-1) early on ACT (before tanh(k) queue),
        # transposes after the hT block so exp has landed by then
        for step in range(NTB + 2):
            if step < 3 and 2 * step + 2 < NTB:
                emit_dma_xT(2 * step + 2, 2)
            if 1 <= step <= 3 and 2 * step < NTB:
                emit_dma_xn(2 * step, 2)
            if 1 <= step <= NTB:
                emit_sred_exp(step - 1)
                if step == NTB:
                    nc.sync.dma_start(e_d[:], e_row[:])
            if step < NTB:
                emit_scores_mm(step)
            if step >= 2:
                emit_cumsum(step - 2)
            if 1 <= step <= NTB:
                emit_etransp(step - 1)
                emit_ue_builds(step - 1)
    return nc


_NC_CACHE: dict[float, object] = {}


def _get_nc(b2: float):
    if b2 not in _NC_CACHE:
        _NC_CACHE[b2] = build_nc(b2)
    return _NC_CACHE[b2]


def _in_maps(x, W1, w2):
    import ml_dtypes

    bf = ml_dtypes.bfloat16
    # ec-major: w1_arr[p, ec, dc, e] = W1[128*dc + p, 128*ec + e]
    w1_arr = np.ascontiguousarray(
        np.asarray(W1, dtype=np.float32)
        .reshape(NC_, P, NC_, P)
        .transpose(1, 2, 0, 3),
        dtype=bf,
    )
    w2_arr = np.ascontiguousarray(
        np.asarray(w2, dtype=np.float32).reshape(NC_, P).T, dtype=bf
    )
    u128 = np.triu(np.ones((P, P), dtype=np.float32)).astype(bf)
    maps = []
    for b in range(B):
        xb = np.asarray(x[b], dtype=np.float32)
        xT_arr = np.ascontiguousarray(
            xb.reshape(NTB, TB, NC_, P).transpose(3, 0, 2, 1), dtype=bf
        )
        xn_arr = np.ascontiguousarray(
            xb.reshape(NM, P, D).transpose(1, 0, 2), dtype=bf
        )
        maps.append(
            {"xT": xT_arr, "xn": xn_arr, "w1": w1_arr, "w2": w2_arr, "u128": u128}
        )
    return maps


def kernel(x, W1, b1, w2, b2, _trace=False, _trace_cores=None):
    import ml_dtypes

    x = np.asarray(x)
    assert x.shape == (B, T, D), x.shape
    assert not np.any(np.asarray(b1)), "b1 != 0 not supported by this build"
    nc = _get_nc(float(np.asarray(b2)))
    res = run_bass_kernel_spmd(
        nc,
        _in_maps(x, W1, w2),
        core_ids=list(range(N_CORES)),
        trace=_trace,
        trace_cores=_trace_cores,
    )
    out = np.empty((B, T, D), dtype=np.float32)
    for b in range(N_CORES):
        num = np.asarray(res.results[b]["num"], dtype=np.float32)  # [P, NM, D]
        e = np.asarray(res.results[b]["e"], dtype=np.float32)[0]   # [T]
        # match the device's bf16 rounding of e inside U_e
        e = e.astype(ml_dtypes.bfloat16).astype(np.float32)
        num = num.transpose(1, 0, 2)                # [NM, P(j), D]
        totals = num[:, P - 1, :]                   # [NM, D]
        carry = np.cumsum(totals, axis=0) - totals  # exclusive
        den = np.cumsum(e).reshape(NM, P)
        out[b] = ((num + carry[:, None, :]) / den[:, :, None]).reshape(T, D)
    if _trace:
        return out, res
    return out


# revision 48
# speedup vs baseline: 1.0198x; 1.0198x over previous
"""Trainium2 Bass kernel for nn_Attention_59785944760577 (sparse_attention).

reference math per batch sample (B=8, one sample per NeuronCore):
  s[t]   = w2 . tanh(x[t] @ W1 + b1) + b2
  e[t]   = exp(s[t])            (softmax shift cancels in the num/den ratio)
  ctx[t] = cumsum_t(e * x) / cumsum_t(e)

v5 pipeline — scores in transposed layout, cumsum via e-scaled triangular
matmuls in natural layout, cross-tile carry + den + divide on the HOST:
  1. host uploads xT bf16 [128p, 8tb, 4dc, 512t] and x-natural tiles
     xn bf16 [128j, 32m, 512d]  (t = 128 m + j)
  2. hT[e,t] = tanh(sum_d W1[d,e] xT[d,t])      PE (W1 chunks stationary)
  3. s[1,t]  = sum_e w2[e] hT[e,t]              PE K-reduction matmul
     e_row   = exp(s + b2)  (fp32)              ACT, single partition
  4. e-cols: PE transpose of e_row 128-slices -> pEcol PSUM -> ecols SBUF
  5. U_e[m]  = U * e_col[m]  (DVE tensor_scalar, bf16)
     pN[m]   = U_e[m]^T xn[m]                   PE: local cumsum of e*x
  6. evict pN -> bf16 (DVE/GPSIMD), DMA out; ship e_row too
  7. HOST: carry[m] = cumsum of tile totals (= row 127 of each tile,
     already in the output), den = cumsum(bf16(e)), out=(local+carry)/den,
     reshaped to [T, D] fp32.
"""
import json
from contextlib import ExitStack

import numpy as np

import concourse.bass as bass
import concourse.tile as tile
from concourse import mybir
from concourse.bass_utils import run_bass_kernel_spmd
from concourse.vector_clock import ScopedClock

F32 = mybir.dt.float32
BF16 = mybir.dt.bfloat16
AF = mybir.ActivationFunctionType
ALU = mybir.AluOpType

B, T, D = 8, 4096, 512
P = 128
NTB = 8          # t-blocks of 512 for the scores phase
TB = T // NTB    # 512
NC_ = 4          # d/e chunks of 128
NM = T // P      # 32 cumsum tiles of 128 rows
N_CORES = 8
N_GPS_EVICT = 12  # how many of the 32 evicts go to gpsimd


# --- workarounds for this walrus build: at most ONE semaphore wait per
# instruction (see baseline notes).
def _patched_drain_and_barrier(self, tick_clock, wait_clock):
    nc = self.nc
    drain_inst = nc.sync.drain()
    wait_clock.add_sem_waits(
        drain_inst.ins, ScopedClock({None: tick_clock.global_clock})
    )
    si = drain_inst.ins.sync_info
    if si is not None and si.on_wait and len(si.on_wait) > 1:
        waits = list(si.on_wait)
        drain_inst.ins.sync_info = mybir.SyncInfo(
            on_wait=waits[:1], on_update=list(si.on_update)
        )
        for w in waits[1:]:
            extra = nc.sync.drain()
            extra.ins.sync_info = mybir.SyncInfo(on_wait=[w], on_update=[])
    nc.all_engine_barrier()
    assert self.sems is not None
    popped = nc._tile_sem_poison_stack.pop()
    assert popped is self._sem_poison
    nc.clear_and_free_semaphores(list(self.sems.allocated().values()))
    nc.all_engine_barrier()


def _split_multiwait_json(data: bytes) -> bytes:
    d = json.loads(data)
    changed = False
    for fn in d.get("functions", []):
        for bb in fn.get("blocks", []):
            new_insts = []
            for inst in bb.get("instructions", []):
                si = inst.get("sync_info")
                waits = si.get("on_wait") if si else None
                if waits and len(waits) > 1:
                    for k, w in enumerate(waits[:-1]):
                        new_insts.append(
                            {
                                "debug": inst.get("debug", 0),
                                "engine": inst["engine"],
                                "ins": [],
                                "outs": [],
                                "name": f"{inst['name']}-ws{k}",
                                "opcode": "NoOp",
                                "sync_info": {"on_update": [], "on_wait": [w]},
                            }
                        )
                    si["on_wait"] = [waits[-1]]
                    changed = True
                new_insts.append(inst)
            if changed:
                bb["instructions"] = new_insts
    return json.dumps(d).encode() if changed else data


def _install_patches():
    if not getattr(tile.TileContext, "_drain_patched", False):
        tile.TileContext._drain_and_barrier = _patched_drain_and_barrier
        tile.TileContext._drain_patched = True
    if not getattr(bass.Bass, "_json_waitsplit_patched", False):
        orig = bass.Bass.to_json_bytes

        def to_json_bytes(self):
            return _split_multiwait_json(orig(self))

        bass.Bass.to_json_bytes = to_json_bytes
        bass.Bass._json_waitsplit_patched = True


def build_nc(b2: float = 0.0):
    _install_patches()
    nc = bass.Bass()
    xT_d = nc.dram_tensor("xT", [P, NTB, NC_, TB], BF16, kind="ExternalInput")
    xn_d = nc.dram_tensor("xn", [P, NM, D], BF16, kind="ExternalInput")
    w1_d = nc.dram_tensor("w1", [P, NC_, NC_, P], BF16, kind="ExternalInput")
    w2_d = nc.dram_tensor("w2", [P, NC_], BF16, kind="ExternalInput")
    u_d = nc.dram_tensor("u128", [P, P], BF16, kind="ExternalInput")
    num_d = nc.dram_tensor("num", [P, NM, D], BF16, kind="ExternalOutput")
    e_d = nc.dram_tensor("e", [1, T], F32, kind="ExternalOutput")

    with tile.TileContext(nc) as tc, ExitStack() as ctx:
        consts = ctx.enter_context(tc.tile_pool(name="consts", bufs=1))
        xpool = ctx.enter_context(tc.tile_pool(name="x", bufs=1))
        hpool = ctx.enter_context(tc.tile_pool(name="h", bufs=2))
        spool = ctx.enter_context(tc.tile_pool(name="s", bufs=1))
        uepool = ctx.enter_context(tc.tile_pool(name="ue", bufs=9))
        obpool = ctx.enter_context(tc.tile_pool(name="ob", bufs=4))
        # PSUM (8 banks): H 3 + S 1 + Ecol 1 + N 3
        psH = ctx.enter_context(tc.tile_pool(name="psH", bufs=3, space="PSUM"))
        psS = ctx.enter_context(tc.tile_pool(name="psS", bufs=1, space="PSUM"))
        psE = ctx.enter_context(tc.tile_pool(name="psE", bufs=1, space="PSUM"))
        psN = ctx.enter_context(tc.tile_pool(name="psN", bufs=3, space="PSUM"))

        w1_sb = consts.tile([P, NC_, NC_, P], BF16, tag="w1")
        w2_sb = consts.tile([P, NC_], BF16, tag="w2")
        u_sb = consts.tile([P, P], BF16, tag="u")
        i11 = consts.tile([1, 1], F32, tag="i11")
        xT = xpool.tile([P, NTB, NC_, TB], BF16, tag="xT")
        xn = xpool.tile([P, NM, D], BF16, tag="xn")

        def emit_dma_xT(tb0, ntb):
            nc.sync.dma_start(xT[:, tb0 : tb0 + ntb], xT_d[:, tb0 : tb0 + ntb])

        def emit_dma_xn(tb0, ntb):
            nc.sync.dma_start(
                xn[:, 4 * tb0 : 4 * (tb0 + ntb), :],
                xn_d[:, 4 * tb0 : 4 * (tb0 + ntb), :],
            )

        # dispatch order tuned for ramp: first-needed chunks first
        # (w1 is ec-major so [:, 0] covers the first ec block's 4 dc chunks)
        nc.sync.dma_start(w1_sb[:, 0], w1_d[:, 0])
        nc.sync.dma_start(xT[:, 0, 0:1], xT_d[:, 0, 0:1])
        nc.sync.dma_start(xT[:, 0, 1:4], xT_d[:, 0, 1:4])
        nc.sync.dma_start(w1_sb[:, 1:4], w1_d[:, 1:4])
        nc.sync.dma_start(xT[:, 1], xT_d[:, 1])
        nc.sync.dma_start(w2_sb[:], w2_d[:])
        nc.sync.dma_start(u_sb[:], u_d[:])
        nc.vector.memset(i11[:], 1.0)
        nc.sync.dma_start(xn[:, 0:4, :], xn_d[:, 0:4, :])
        nc.sync.dma_start(xn[:, 4:8, :], xn_d[:, 4:8, :])

        e_row = spool.tile([1, T], F32, tag="erow")
        ecols = spool.tile([P, NM], F32, tag="ecols")
        pEcol = psE.tile([P, NM], F32, tag="pecol")

        hts = [None] * NTB
        pss = [None] * NTB

        def emit_scores_mm(tb):
            ht = hpool.tile([P, NC_, TB], BF16)
            hts[tb] = ht
            for ec in range(NC_):
                ph = psH.tile([P, TB], F32)
                for dc in range(NC_):
                    nc.tensor.matmul(
                        ph[:],
                        w1_sb[:, ec, dc, :],
                        xT[:, tb, dc, :],
                        start=(dc == 0),
                        stop=(dc == NC_ - 1),
                    )
                nc.scalar.activation(ht[:, ec, :], ph[:], AF.Tanh)

        def emit_sred_exp(tb):
            ps = psS.tile([1, TB], F32)
            pss[tb] = ps
            ht = hts[tb]
            for ec in range(NC_):
                nc.tensor.matmul(
                    ps[:],
                    w2_sb[:, ec : ec + 1],
                    ht[:, ec, :],
                    start=(ec == 0),
                    stop=(ec == NC_ - 1),
                )
            nc.scalar.activation(
                e_row[0:1, tb * TB : (tb + 1) * TB], ps[:], AF.Exp, bias=float(b2)
            )

        def emit_etransp(tb):
            # 4 transposes: e_row [1,128] slices -> pEcol columns, then one evict
            for j in range(4):
                m = 4 * tb + j
                nc.tensor.transpose(
                    pEcol[:, m : m + 1], e_row[0:1, m * P : (m + 1) * P], i11[:]
                )
            nc.vector.tensor_copy(
                ecols[:, 4 * tb : 4 * tb + 4], pEcol[:, 4 * tb : 4 * tb + 4]
            )

        ue_tiles = {}

        def emit_ue_builds(tb):
            for j in range(4):
                m = 4 * tb + j
                ue = uepool.tile([P, P], BF16)
                nc.vector.tensor_scalar_mul(ue[:], u_sb[:], ecols[:, m : m + 1])
                ue_tiles[m] = ue

        def emit_cumsum(tb):
            last = tb == NTB - 1
            ob = obpool.tile([P, 4, D], BF16)
            for j in range(4):
                m = 4 * tb + j
                pn = psN.tile([P, D], F32)
                nc.tensor.matmul(
                    pn[:], ue_tiles.pop(m)[:], xn[:, m, :], start=True, stop=True
                )
                if last:
                    # split halves across ACT+DVE: half the evict latency
                    nc.scalar.copy(ob[:, j, 0 : D // 2], pn[:, 0 : D // 2])
                    nc.vector.tensor_copy(ob[:, j, D // 2 : D], pn[:, D // 2 : D])
                elif j % 2 == 0:
                    nc.scalar.copy(ob[:, j, :], pn[:])
                else:
                    nc.vector.tensor_copy(ob[:, j, :], pn[:])
                if last and j % 2 == 1:
                    nc.sync.dma_start(
                        num_d[:, m - 1 : m + 1, :], ob[:, j - 1 : j + 1, :]
                    )
            if not last:
                nc.sync.dma_start(num_d[:, 4 * tb : 4 * (tb + 1), :], ob[:])

        # software pipeline: exp(k-1) early on ACT (before tanh(k) queue),
        # transposes after the hT block so exp has landed by then
        for step in range(NTB + 2):
            if step < 3 and 2 * step + 2 < NTB:
                emit_dma_xT(2 * step + 2, 2)
            if 1 <= step <= 3 and 2 * step < NTB:
                emit_dma_xn(2 * step, 2)
            if 1 <= step <= NTB:
                emit_sred_exp(step - 1)
                if step == NTB:
                    nc.sync.dma_start(e_d[:], e_row[:])
            if step < NTB:
                emit_scores_mm(step)
            if step >= 2:
                emit_cumsum(step - 2)
            if 1 <= step <= NTB:
                emit_etransp(step - 1)
                emit_ue_builds(step - 1)
    return nc


_NC_CACHE: dict[float, object] = {}


def _get_nc(b2: float):
    if b2 not in _NC_CACHE:
        _NC_CACHE[b2] = build_nc(b2)
    return _NC_CACHE[b2]


def _in_maps(x, W1, w2):
    import ml_dtypes

    bf = ml_dtypes.bfloat16
    # ec-major: w1_arr[p, ec, dc, e] = W1[128*dc + p, 128*ec + e]
    w1_arr = np.ascontiguousarray(
        np.asarray(W1, dtype=np.float32)
        .reshape(NC_, P, NC_, P)
        .transpose(1, 2, 0, 3),
        dtype=bf,
    )
    w2_arr = np.ascontiguousarray(
        np.asarray(w2, dtype=np.float32).reshape(NC_, P).T, dtype=bf
    )
    u128 = np.triu(np.ones((P, P), dtype=np.float32)).astype(bf)
    maps = []
    for b in range(B):
        xb = np.asarray(x[b], dtype=np.float32)
        xT_arr = np.ascontiguousarray(
            xb.reshape(NTB, TB, NC_, P).transpose(3, 0, 2, 1), dtype=bf
        )
        xn_arr = np.ascontiguousarray(
            xb.reshape(NM, P, D).transpose(1, 0, 2), dtype=bf
        )
        maps.append(
            {"xT": xT_arr, "xn": xn_arr, "w1": w1_arr, "w2": w2_arr, "u128": u128}
        )
    return maps


def kernel(x, W1, b1, w2, b2, _trace=False, _trace_cores=None):
    import ml_dtypes

    x = np.asarray(x)
    assert x.shape == (B, T, D), x.shape
    assert not np.any(np.asarray(b1)), "b1 != 0 not supported by this build"
    nc = _get_nc(float(np.asarray(b2)))
    res = run_bass_kernel_spmd(
        nc,
        _in_maps(x, W1, w2),
        core_ids=list(range(N_CORES)),
        trace=_trace,
        trace_cores=_trace_cores,
    )
    out = np.empty((B, T, D), dtype=np.float32)
    for b in range(N_CORES):
        num = np.asarray(res.results[b]["num"], dtype=np.float32)  # [P, NM, D]
        e = np.asarray(res.results[b]["e"], dtype=np.float32)[0]   # [T]
        # match the device's bf16 rounding of e inside U_e
        e = e.astype(ml_dtypes.bfloat16).astype(np.float32)
        num = num.transpose(1, 0, 2)                # [NM, P(j), D]
        totals = num[:, P - 1, :]                   # [NM, D]
        carry = np.cumsum(totals, axis=0) - totals  # exclusive
        den = np.cumsum(e).reshape(NM, P)
        out[b] = ((num + carry[:, None, :]) / den[:, :, None]).reshape(T, D)
    if _trace:
        return out, res
    return out
